# revision 56
# baseline (speedup 1.0000x reference)
"""Bilateral denoiser Trainium2 kernel (8 NeuronCores, data-parallel over H).

Algorithm (per core, H-slice of 28 rows x all 6 images), using the identity
  num[x] = P[x]*den[x] + U[x] - V[x],   out = P + (U - V)/den
where for each symmetric offset pair k=(dy,dx) (dy>0, or dy==0 & dx>0):
  d_k[x] = P[x+k] - P[x]
  w_k[x] = exp(-d_k^2/ds^2 - |k|^2/bs^2)
  M_k    = w_k * d_k          (ONE product per pair)
  den    = 1 + sum_k (w_k[x] + w_k[x-k])          (PE: +shift matmuls)
  U - V  = sum_k (M_k[x] - M_k[x-k])              (PE: +/- shift matmuls)

Engine split: DVE fp16 subtract (one op per parity half) + M product +
~60% of squares; ACT ~40% of squares + per-pair-bias exp (same-|dx| pairs
stacked); PE (FWL-enabled 128-col weights) accumulates den and U-V in PSUM
via (+/-) shifted identity matmuls over junk-trimmed 448-col AP views,
with a HAM warmup stream during pipeline fill.
"""

import numpy as np

# ---- problem constants (hardcoded per contract) ----
B, C, H, W = 2, 3, 224, 224
NIMG = B * C          # 6
NCORES = 8
CR = H // NCORES      # 28 output rows per core
PAD = 7               # filter 15 -> halo 7
SEGS, GRPS = 3, 2     # images: 3 on partitions x 2 on free dim
SROWS = CR + 2 * PAD  # 42 rows per segment
PARTS = SEGS * SROWS  # 126 partitions of P tile
GW = W + 2 * PAD      # 238 padded cols per group
GUARD = 14
PCOLS = GUARD + GRPS * GW + GUARD  # 504
SLICE_G = 252         # per-group cols in a stack slice
SLICE_W = GRPS * SLICE_G  # 504 free elems per k-slice
CPART = PARTS - PAD   # 119: compute-partition count
MPART = 112           # real output partitions
MLHS = 128            # lhs cols padded to 128 (enables FWL fast weight load)
POUT = 478            # psum: [pad 1][g0 224][junk 28][g1 224][pad 1]
TRIM = 240            # elementwise cols per group actually consumed [6,246)
PADVAL = -100.0

_CACHE = {}


def _pairs():
    """(dy, [dx...]) groups with dy>0, or dy==0 and dx>0.  dxs ordered
    odd-first then even: uniform stride 2 within each parity half (one
    subtract AP each), +/-dx pairs (equal blur bias) adjacent-symmetric
    within each half."""
    out = []
    for dy in range(0, PAD + 1):
        dxs = [dx for dx in range(-PAD, PAD + 1) if (dy > 0 or dx > 0)]
        odd = [dx for dx in dxs if dx % 2 != 0]
        even = [dx for dx in dxs if dx % 2 == 0]
        if dy <= 1:
            # fine-grained chunks at the start: faster pipeline fill
            out.append((dy, odd, len(odd)))
            out.append((dy, even, 0))
        else:
            out.append((dy, odd + even, len(odd)))
    return out


def _sq_split(n):
    """Slice counts (gpsimd, act, dve) for the square of an n-slice group."""
    import os
    fg = float(os.environ.get("BILAT_SQ_GPS", "0.0"))
    fa = float(os.environ.get("BILAT_SQ_ACT", "0.34"))
    ng = int(round(n * fg))
    na = int(round(n * fa))
    nd = n - ng - na
    assert nd >= 0
    return ng, na, nd


def _build(inv_d, inv_b):
    import concourse.bacc as bacc
    import concourse.mybir as mybir
    import concourse.tile as tile
    import bass_rust
    from concourse.tile import add_dep_helper
    from contextlib import ExitStack

    dt = mybir.dt
    F32, F16 = dt.float32, dt.float16
    ALU = mybir.AluOpType
    AF = mybir.ActivationFunctionType

    groups = _pairs()
    npairs = sum(len(dxs) for _, dxs, _ in groups)  # 112

    nc = bacc.Bacc("TRN2", target_bir_lowering=False, debug=False,
                   num_devices=NCORES)

    xin = nc.dram_tensor("xin", [PARTS, PCOLS], F32, kind="ExternalInput").ap()
    xin16 = nc.dram_tensor("xin16", [PARTS, PCOLS], F16,
                           kind="ExternalInput").ap()
    id_ext = nc.dram_tensor("shmat", [CPART, 16 * MLHS], F16,
                            kind="ExternalInput").ap()
    bt_ext = nc.dram_tensor("btab", [128, npairs], F32,
                            kind="ExternalInput").ap()
    y_ext = nc.dram_tensor("y", [MPART, POUT], F32, kind="ExternalOutput").ap()

    def mk(t, npart, pstart, free_pairs, coloff):
        """Custom AP over tile t: partitions [pstart, pstart+npart) plus
        explicit free (step,count) pairs with element offset coloff."""
        assert t.offset == 0, t.offset
        pitch = t.ap[0][0]
        a = t.copy()
        a.ap = bass_rust.VecI64Pair([(pitch, npart)] + list(free_pairs))
        a.offset = int(pstart * pitch + coloff)
        return a

    tail = []  # producers the final drain must observe

    with tile.TileContext(nc) as tc:
        with ExitStack() as ctx:
            const = ctx.enter_context(tc.tile_pool(name="const", bufs=1))
            tpool = ctx.enter_context(tc.tile_pool(name="tp", bufs=5))
            wpool = ctx.enter_context(tc.tile_pool(name="wp", bufs=4))
            spool = ctx.enter_context(tc.tile_pool(name="sp", bufs=3))
            ppool = ctx.enter_context(tc.tile_pool(name="pp", bufs=3))
            fin = ctx.enter_context(tc.tile_pool(name="fin", bufs=1))
            psum = ctx.enter_context(tc.tile_pool(name="ps", bufs=1,
                                                  space="PSUM"))

            # ---- constants / input staging, in dependency-urgency order:
            # Pe/Po gate the first subtract, btab gates the first exp (keep
            # it AHEAD of the big 487KB sh16 load), sh16 gates matmuls,
            # Pcent only gates the epilogue ----
            Pe = const.tile([PARTS, PCOLS], F16)
            nc.sync.dma_start(Pe[:], xin16[:])
            Po = const.tile([PARTS, PCOLS], F16)
            nc.sync.dma_start(Po[:, 0:PCOLS - 1], xin16[:, 1:PCOLS])
            btab = const.tile([128, npairs], F32)
            nc.sync.dma_start(btab[:], bt_ext[:])
            sh16 = const.tile([CPART, 16 * MLHS], F16)
            nc.sync.dma_start(sh16[:], id_ext[:])
            ones = const.tile([CPART, POUT], F16)
            nc.gpsimd.memset(ones[:], 1.0)
            # partition-aligned center-P copy in output layout (for epilogue)
            Pcent = const.tile([MPART, POUT], F32)
            nc.sync.dma_start(
                mk(Pcent, MPART, 0, [(252, GRPS), (1, W)], 1),
                mk(xin, MPART, PAD, [(GW, GRPS), (1, W)], GUARD + PAD))

            pd = psum.tile([MLHS, POUT], F32)
            pn = psum.tile([MLHS, POUT], F32)

            shU = sh16[:, 7 * MLHS:8 * MLHS]

            # HAM warmup: keep PE busy while the first group's elementwise
            # chain fills the pipeline (results land in a scratch bank)
            import os
            nwarm = int(os.environ.get("BILAT_WARM", "4"))
            if nwarm:
                warm = psum.tile([MLHS, POUT], F32)
            for _ in range(nwarm):
                nc.tensor.matmul(mk(warm, MLHS, 0, [(1, 448)], 1),
                                 shU, mk(ones, CPART, 0, [(1, 448)], 1),
                                 start=True, stop=True, skip_group_check=True)

            # seed den with the center 1.0 (opens pd accumulation)
            mm = nc.tensor.matmul(mk(pd, MLHS, 0, [(1, POUT)], 0),
                                  shU, mk(ones, CPART, 0, [(1, POUT)], 0),
                                  start=True, stop=False)

            n_mm = 1
            total_mm = 1 + 4 * npairs
            pn_started = False
            pair_idx = 0

            def rhs_out(tile_, b, ps):
                """Junk-trimmed rhs/out pair: exactly the 2x224 real cols."""
                return (mk(tile_, CPART, 0, [(252, 2), (1, 224)], b),
                        mk(ps, MLHS, 0, [(252, 2), (1, 224)], 1))

            cur_dy = -1
            pending = None
            for dy, dxs, nodd in groups:
                Kc = len(dxs)
                if dy == 0:
                    Pedy, Pody = Pe, Po
                elif dy != cur_dy:
                    Pedy = ppool.tile([CPART, PCOLS], F16, tag="Pedy")
                    nc.sync.dma_start(Pedy[:], xin16[dy:dy + CPART, :])
                    Pody = ppool.tile([CPART, PCOLS], F16, tag="Pody")
                    nc.sync.dma_start(Pody[:, 0:PCOLS - 1],
                                      xin16[dy:dy + CPART, 1:PCOLS])
                cur_dy = dy

                T = tpool.tile([CPART, Kc * SLICE_W], F16, tag="T",
                               padded_shape=[CPART, 15 * SLICE_W])
                Wt = wpool.tile([CPART, Kc * SLICE_W], F16, tag="W",
                                padded_shape=[CPART, 15 * SLICE_W])
                Mt = spool.tile([CPART, Kc * SLICE_W], F16, tag="M",
                                padded_shape=[CPART, 15 * SLICE_W])

                # ---- d = P(+dy,+dx) - P, fp16 2x, one op per parity half
                # (each half has uniform dx stride 2) ----
                for j0, kn in ((0, nodd), (nodd, Kc - nodd)):
                    if not kn:
                        continue
                    dx0 = dxs[j0]
                    if dx0 % 2 != 0:
                        src, sb = Pedy, 7 + dx0
                    else:
                        src, sb = Pody, 6 + dx0
                    in0 = mk(src, CPART, 0,
                             [(2, kn), (GW, GRPS), (1, TRIM)], sb + 6)
                    in1 = mk(Po, CPART, 0,
                             [(0, kn), (GW, GRPS), (1, TRIM)], 12)
                    outT = mk(T, CPART, 0,
                              [(SLICE_W, kn), (SLICE_G, GRPS),
                               (1, TRIM)], j0 * SLICE_W + 6)
                    nc.vector.tensor_tensor(outT, in0, in1, ALU.subtract)

                # ---- square: split across gpsimd / ACT / DVE ----
                ng, na, nd = _sq_split(Kc)

                def tv(tile_, c0, n):
                    return mk(tile_, CPART, 0,
                              [(SLICE_W, n), (SLICE_G, GRPS), (1, TRIM)],
                              c0 * SLICE_W + 6)
                c0 = 0
                if ng:
                    nc.gpsimd.tensor_tensor(tv(Wt, c0, ng), tv(T, c0, ng),
                                            tv(T, c0, ng), ALU.mult)
                    c0 += ng
                if na:
                    nc.scalar.activation(tv(Wt, c0, na), tv(T, c0, na),
                                         AF.Square, bias=0.0, scale=1.0)
                    c0 += na
                if nd:
                    nc.vector.tensor_tensor(tv(Wt, c0, nd), tv(T, c0, nd),
                                            tv(T, c0, nd), ALU.mult)

                # ---- exp in place, per-pair blur bias; same-|dx| pairs
                # share a bias so stack them in one op ----
                done = [False] * Kc
                for j in range(Kc):
                    if done[j]:
                        continue
                    jj = pair_idx + j
                    j2 = dxs.index(-dxs[j]) if -dxs[j] in dxs else -1
                    if j2 > j:
                        ap = mk(Wt, CPART, 0,
                                [((j2 - j) * SLICE_W, 2), (SLICE_G, GRPS),
                                 (1, TRIM)], j * SLICE_W + 6)
                        done[j2] = True
                    else:
                        ap = mk(Wt, CPART, 0,
                                [(SLICE_G, GRPS), (1, TRIM)],
                                j * SLICE_W + 6)
                    nc.scalar.activation(ap, ap, AF.Exp,
                                         bias=btab[0:CPART, jj:jj + 1],
                                         scale=-float(inv_d))
                    done[j] = True

                # ---- M-mult + matmuls are emitted one group LATE so the
                # M product (which waits on this group's exp) never blocks
                # the next group's subtract in the DVE FIFO ----
                def flush(T=T, Wt=Wt, Mt=Mt, dy=dy, dxs=dxs, Kc=Kc,
                          nodd=nodd):
                    nonlocal n_mm, pn_started, mm
                    shS = sh16[:, (7 - dy) * MLHS:(8 - dy) * MLHS]
                    shSn = sh16[:, (15 - dy) * MLHS:(16 - dy) * MLHS]

                    def emit(seq):
                        nonlocal n_mm, pn_started, mm
                        for lh, (rhs, outv), is_pn in seq:
                            n_mm += 1
                            st = is_pn and not pn_started
                            if st:
                                pn_started = True
                            mm = nc.tensor.matmul(outv, lh, rhs,
                                                  start=st,
                                                  stop=(n_mm == total_mm))

                    seq = []
                    for j, dx in enumerate(dxs):
                        seq.append((shU, rhs_out(Wt, j * SLICE_W + GUARD,
                                                 pd), False))
                    for j, dx in enumerate(dxs):
                        seq.append((shS, rhs_out(Wt,
                                                 j * SLICE_W + GUARD - dx,
                                                 pd), False))
                    emit(seq)
                    # M = W * d per parity half, matmuls right behind each
                    # half so the PE isn't gated on the full product
                    for j0, kn in ((0, nodd), (nodd, Kc - nodd)):
                        if not kn:
                            continue
                        def tv2(tile_):
                            return mk(tile_, CPART, 0,
                                      [(SLICE_W, kn), (SLICE_G, GRPS),
                                       (1, TRIM)], j0 * SLICE_W + 6)
                        nc.vector.tensor_tensor(tv2(Mt), tv2(Wt), tv2(T),
                                                ALU.mult)
                        seq = []
                        for j in range(j0, j0 + kn):
                            seq.append((shU, rhs_out(Mt,
                                                     j * SLICE_W + GUARD,
                                                     pn), True))
                        for j in range(j0, j0 + kn):
                            seq.append((shSn,
                                        rhs_out(Mt, j * SLICE_W + GUARD
                                                - dxs[j], pn), True))
                        emit(seq)

                if pending is not None:
                    pending()
                pending = flush
                pair_idx += Kc
            if pending is not None:
                pending()

            # ---- finale: out = P + (U - V) / den ----
            rec = fin.tile([MPART, POUT], F32)
            rc = nc.vector.reciprocal_approx_fast(rec[:], pd[0:MPART, :])
            tt = fin.tile([MPART, POUT], F32)
            fm = nc.vector.tensor_tensor(tt[:], pn[0:MPART, :], rec[:], ALU.mult)
            outt = fin.tile([MPART, POUT], F32)
            Pc = mk(Pcent, MPART, 0, [(252, GRPS), (1, W)], 1)
            to = mk(tt, MPART, 0, [(252, GRPS), (1, W)], 1)
            oo = mk(outt, MPART, 0, [(252, GRPS), (1, W)], 1)
            fa = nc.vector.tensor_tensor(oo, to, Pc, ALU.add)
            dout = nc.sync.dma_start(y_ext[:], outt[:])
            tail += [mm, rc, fm, fa, dout]

            for prod in tail:
                n = nc.sync.nop()
                add_dep_helper(n.ins, prod.ins, sync=True,
                               reason="drain fanin")

    nc.compile()
    return nc


def _prep_inputs(x, inv_b):
    """x: [B,C,H,W] fp32 -> per-core staged arrays + constants."""
    xi = x.reshape(NIMG, H, W).astype(np.float32)
    Pg = np.full((NIMG, H + 2 * PAD, W + 2 * PAD), PADVAL, np.float32)
    Pg[:, PAD:PAD + H, PAD:PAD + W] = xi

    groups = _pairs()
    npairs = sum(len(dxs) for _, dxs, _ in groups)
    btab = np.zeros((128, npairs), np.float32)
    i = 0
    for dy, dxs, _ in groups:
        for dx in dxs:
            btab[:, i] = -(dy * dy + dx * dx) * inv_b
            i += 1
    # 8 positive shift matrices (d = shift amount), then 8 negated copies
    shmat = np.zeros((CPART, 16 * MLHS), np.float32)
    for d in range(8):
        for m in range(MPART):
            shmat[m + d, d * MLHS + m] = 1.0
            shmat[m + d, (8 + d) * MLHS + m] = -1.0

    maps = []
    for c in range(NCORES):
        arr = np.full((PARTS, PCOLS), PADVAL, np.float32)
        r0 = c * CR  # strip top in padded-row coords
        for s in range(SEGS):
            for g in range(GRPS):
                m = g * SEGS + s
                arr[s * SROWS:(s + 1) * SROWS,
                    GUARD + g * GW:GUARD + (g + 1) * GW] = \
                    Pg[m, r0:r0 + SROWS, :]
        maps.append({"xin": arr, "xin16": arr.astype(np.float16),
                     "shmat": shmat.astype(np.float16), "btab": btab})
    return maps


def kernel(x, blur_sigma, diff_sigma, filter_size):
    x = np.asarray(x, dtype=np.float32)
    assert x.shape == (B, C, H, W)
    assert int(filter_size) == 15
    inv_d = 1.0 / float(diff_sigma) ** 2
    inv_b = 1.0 / float(blur_sigma) ** 2

    import os
    key = (round(inv_d, 12), round(inv_b, 12),
           os.environ.get("BILAT_SQ_GPS", "0.0"),
           os.environ.get("BILAT_SQ_ACT", "0.34"))
    if key not in _CACHE:
        _CACHE[key] = _build(inv_d, inv_b)
    nc = _CACHE[key]

    from concourse.bass_utils import run_bass_kernel_spmd
    maps = _prep_inputs(x, inv_b)
    kw = {}
    if int(os.environ.get("BILAT_TRACE", "0")):
        kw = dict(trace=True)
    res = run_bass_kernel_spmd(nc, maps, list(range(NCORES)), **kw)
    global _LAST_EXEC_NS
    _LAST_EXEC_NS = res.exec_time_ns

    out = np.empty((NIMG, H, W), np.float32)
    for c in range(NCORES):
        y = res.results[c]["y"]  # [112, 478]
        for s in range(SEGS):
            for g in range(GRPS):
                m = g * SEGS + s
                out[m, c * CR:(c + 1) * CR, :] = \
                    y[s * SROWS:s * SROWS + CR, 1 + g * 252:1 + g * 252 + W]
    return out.reshape(B, C, H, W)


_LAST_EXEC_NS = None
